# revision 4
# baseline (speedup 1.0000x reference)
"""Trainium2 Bass kernel: negative squared-distance VQ codebook scores.

score[b,t,k] = -precision * ||x[b,t] - codebook[k]||^2
             = 2p * (x.c) - p * ||x||^2 - p * ||c||^2

Strategy (8 NeuronCores, data-parallel over B):
  - Each core gets 2 batches = 2048 (b,t) rows of x; codebook replicated.
  - On-chip: transpose x tiles (PE) to put D on partitions, scale by -2.
  - GEMM in float32r (full-rate fp32 mode): psum = -2*x.c, with ||c||^2
    folded in as a rank-1 update (ones ⊗ c_sq row).
  - Epilogue: out = (-p) * psum + (-p * x_sq)  per-partition scalars,
    alternated between ScalarE (activation) and VectorE (tensor_scalar).
"""

from contextlib import ExitStack

import numpy as np

import concourse.bass as bass
import concourse.tile as tile
from concourse import bacc, mybir
from concourse.bass_utils import run_bass_kernel_spmd
from concourse.masks import make_identity

B, T, D, K = 16, 1024, 256, 1024
N_CORES = 8
BT = B * T // N_CORES  # rows of x per core (2048)
P = 128                # partition tile
NT = BT // P           # bt tiles per core (16)
KH = 512               # k slice per matmul (fp32 moving-operand max)
DH = D // P            # d halves (2)
KT = K // P            # codebook row tiles (8)

F32 = mybir.dt.float32
F32R = mybir.dt.float32r
AF = mybir.ActivationFunctionType
OP = mybir.AluOpType


def _build_kernel(ctx: ExitStack, tc: tile.TileContext, x_in, cb_in, p_in, out):
    nc = tc.nc

    singles = ctx.enter_context(tc.tile_pool(name="singles", bufs=1))
    cbt_pool = ctx.enter_context(tc.tile_pool(name="cbt", bufs=1))

    ident = singles.tile([P, P], F32)
    make_identity(nc, ident)

    # precision broadcast to [128,1]; neg_p = -p
    p_bc = singles.tile([P, 1], F32)
    nc.gpsimd.dma_start(out=p_bc, in_=p_in.to_broadcast([P, 1]))
    neg_p = singles.tile([P, 1], F32)
    nc.vector.tensor_scalar_mul(neg_p, p_bc, -1.0)

    # memset can't emit fp32r; stage in f32 and copy (ALU ops can round)
    ones_col_f32 = singles.tile([P, 1], F32)
    nc.vector.memset(ones_col_f32, 1.0)
    ones_col = singles.tile([P, 1], F32R)
    nc.vector.tensor_copy(ones_col, ones_col_f32)
    ones_row_f32 = singles.tile([1, P], F32)
    nc.vector.memset(ones_row_f32, 1.0)
    ones_row = singles.tile([1, P], F32R)
    nc.vector.tensor_copy(ones_row, ones_row_f32)

    # c_sq row [1, K] (raw sum of squares of codebook rows)
    csq_sb = singles.tile([1, K], F32R)
    # transposed codebook [d_local, half, k] (raw)
    cbt = cbt_pool.tile([P, DH, K], F32R)

    # ---- preamble: load + transpose codebook, compute c_sq row ----
    with (
        tc.tile_pool(name="pre", bufs=2) as pre,
        tc.tile_pool(name="pre_sq", bufs=1) as pre_sq,
        tc.tile_pool(name="pre_ps", bufs=2, space="PSUM") as pre_ps,
        tc.tile_pool(name="pre_ps_row", bufs=2, space="PSUM") as pre_ps_row,
    ):
        for kt in range(KT):
            cbn = pre.tile([P, D], F32)
            nc.sync.dma_start(out=cbn, in_=cb_in[kt * P : (kt + 1) * P, :])
            for h in range(DH):
                ps_t = pre_ps.tile([P, P], F32)
                nc.tensor.transpose(ps_t, cbn[:, h * P : (h + 1) * P], ident)
                dst = cbt[:, h, kt * P : (kt + 1) * P]
                if (kt + h) % 2 == 0:
                    nc.scalar.copy(dst, ps_t)
                else:
                    nc.vector.tensor_copy(dst, ps_t)

        # squares of cbt, then column-sum via PE with ones weights
        sqc = pre_sq.tile([P, DH, K], F32R)
        nc.scalar.activation(out=sqc[:, 0, :], in_=cbt[:, 0, :], func=AF.Square)
        nc.vector.tensor_mul(sqc[:, 1, :], cbt[:, 1, :], cbt[:, 1, :])
        for kq in range(K // KH):
            ps_c = pre_ps_row.tile([1, KH], F32)
            for h in range(DH):
                nc.tensor.matmul(
                    ps_c,
                    lhsT=ones_col,
                    rhs=sqc[:, h, kq * KH : (kq + 1) * KH],
                    start=(h == 0),
                    stop=(h == DH - 1),
                )
            nc.vector.tensor_copy(csq_sb[:, kq * KH : (kq + 1) * KH], ps_c)

    # ---- main loop over bt tiles ----
    xn_pool = ctx.enter_context(tc.tile_pool(name="xn", bufs=3))
    xt_pool = ctx.enter_context(tc.tile_pool(name="xt", bufs=3))
    dump_pool = ctx.enter_context(tc.tile_pool(name="dump", bufs=2))
    small_pool = ctx.enter_context(tc.tile_pool(name="small", bufs=4))
    out_pool = ctx.enter_context(tc.tile_pool(name="outp", bufs=3))
    ps_t_pool = ctx.enter_context(tc.tile_pool(name="ps_t", bufs=2, space="PSUM"))
    ps_mm_pool = ctx.enter_context(tc.tile_pool(name="ps_mm", bufs=4, space="PSUM"))

    for i in range(NT):
        xn = xn_pool.tile([P, D], F32)
        nc.sync.dma_start(out=xn, in_=x_in[i * P : (i + 1) * P, :])

        # x_sq[bt] = sum_d x^2 via ScalarE square + row-accumulate
        dump = dump_pool.tile([P, D], F32)
        x_sq = small_pool.tile([P, 1], F32)
        nc.scalar.activation(out=dump, in_=xn, func=AF.Square, accum_out=x_sq)
        negp_xsq = small_pool.tile([P, 1], F32)
        nc.vector.tensor_mul(negp_xsq, x_sq, neg_p)

        # transpose x tile, folding in the -2 factor
        xt2 = xt_pool.tile([P, DH, P], F32R)
        for h in range(DH):
            ps_t = ps_t_pool.tile([P, P], F32)
            nc.tensor.transpose(ps_t, xn[:, h * P : (h + 1) * P], ident)
            if h == 0:
                nc.scalar.activation(
                    out=xt2[:, h, :], in_=ps_t, func=AF.Identity, scale=-2.0
                )
            else:
                nc.vector.tensor_scalar_mul(xt2[:, h, :], ps_t, -2.0)

        out_t = out_pool.tile([P, K], F32)
        ps_mms = [
            ps_mm_pool.tile(
                [P, KH], F32, name=f"ps_mm{kq}", tag=f"ps_mm{kq}", bufs=2
            )
            for kq in range(2)
        ]
        for h in range(DH):
            for kq in range(2):
                nc.tensor.matmul(
                    ps_mms[kq],
                    lhsT=xt2[:, h, :],
                    rhs=cbt[:, h, kq * KH : (kq + 1) * KH],
                    start=(h == 0),
                    stop=False,
                )
        for kq in range(2):
            # rank-1: add c_sq to every row
            nc.tensor.matmul(
                ps_mms[kq],
                lhsT=ones_row,
                rhs=csq_sb[:, kq * KH : (kq + 1) * KH],
                start=False,
                stop=True,
            )
        # epilogue: out = neg_p * psum + negp_xsq   (psum = -2 x.c + c_sq)
        nc.scalar.activation(
            out=out_t[:, 0:KH],
            in_=ps_mms[0],
            func=AF.Identity,
            bias=negp_xsq,
            scale=neg_p,
        )
        nc.vector.tensor_scalar(
            out=out_t[:, KH:K],
            in0=ps_mms[1],
            scalar1=neg_p,
            scalar2=negp_xsq,
            op0=OP.mult,
            op1=OP.add,
        )

        nc.sync.dma_start(out=out[i * P : (i + 1) * P, :], in_=out_t)


def build_program():
    nc = bacc.Bacc(
        "TRN2", target_bir_lowering=False, debug=False, num_devices=N_CORES
    )
    x_in = nc.dram_tensor("x", [BT, D], F32, kind="ExternalInput").ap()
    cb_in = nc.dram_tensor("codebook", [K, D], F32, kind="ExternalInput").ap()
    p_in = nc.dram_tensor("precision", [1, 1], F32, kind="ExternalInput").ap()
    out = nc.dram_tensor("out", [BT, K], F32, kind="ExternalOutput").ap()

    with tile.TileContext(nc) as tc:
        with ExitStack() as ctx:
            _build_kernel(ctx, tc, x_in, cb_in, p_in, out)
    nc.compile()
    return nc


_PROGRAM = None


def _get_program():
    global _PROGRAM
    if _PROGRAM is None:
        _PROGRAM = build_program()
    return _PROGRAM


def kernel(x, codebook, precision, _trace=False):
    x = np.ascontiguousarray(np.asarray(x, dtype=np.float32))
    codebook = np.ascontiguousarray(np.asarray(codebook, dtype=np.float32))
    precision = np.ascontiguousarray(np.asarray(precision, dtype=np.float32))
    assert x.shape == (B, T, D) and codebook.shape == (K, D)

    nc = _get_program()
    rows_per_core = B // N_CORES  # 2 batches per core
    in_maps = [
        {
            "x": x[c * rows_per_core : (c + 1) * rows_per_core].reshape(BT, D),
            "codebook": codebook,
            "precision": precision.reshape(1, 1),
        }
        for c in range(N_CORES)
    ]
    res = run_bass_kernel_spmd(
        nc, in_maps, core_ids=list(range(N_CORES)), trace=_trace
    )
    out = np.concatenate(
        [r["out"].reshape(rows_per_core, T, K) for r in res.results], axis=0
    )
    if _trace:
        kernel.last_exec_time_ns = res.exec_time_ns
        kernel.last_results = res
    return out


if __name__ == "__main__":
    xs = np.random.randn(B, T, D).astype(np.float32)
    cb = np.random.randn(K, D).astype(np.float32)
    pr = np.ones((1,), dtype=np.float32)
    o = kernel(xs, cb, pr)
    print(o.shape, o.dtype)
